# revision 2
# baseline (speedup 1.0000x reference)
"""V7: fully-static raw-bass ragged segment mean, prep/trigger pipelined gather.

Strategy (vs V6 baseline):
- Compile happens inside kernel() AFTER seeing begin/end, so the whole
  program is specialized to the actual ragged sizes: LPT-balanced core/group
  assignment, exact static tile/call counts, static num_idxs — no
  value_load, no empty boot-guard calls, no runtime-count path.
- Gathers use dma_gather(prepare_only=True) + trigger_dma so Q7 descriptor
  generation overlaps the SDMA transfers; transfers run back-to-back at the
  HBM-per-core roofline (~360 GB/s). The non-prep path serializes desc-gen
  (994ns + 0.34ns/desc) with each 2MB transfer.
- Raw bass (no TileContext): manual semaphore choreography, engines:
    SP   : const loads (idx/colw/iota), per-group output stores
    Pool : per call: prep (sem=dsem[slot]) -> wait prep-sem -> trigger
    DVE  : all per-tile selection matrices up front, then psum->SBUF drains
    PE   : per tile: psum[group] += sel[128,64].T @ gtile[128,512] (f32r)
- Padding rows gather row 0 of the group region with weight 0 (no -1
  indices: every engine always gets descriptors so completion sems fire,
  and sim traffic == HW traffic).

Per-core bytes ~= 33.9k rows * 2KiB ~= 69.5 MB -> ~195 us transfer floor.
"""

import time

import numpy as np

import concourse.bass as bass
from concourse import bacc
import concourse.mybir as mybir
from concourse.bass_utils import run_bass_kernel_spmd

B, L, D = 2048, 512, 512
NCORES = 8
BL = B // NCORES  # 256
GB = 64  # b's per group (region = GB*L = 32768 rows -> int16 idx max)
GRPS = BL // GB  # 4
CT = 8  # max tiles per dma_gather call (8*128=1024 idx)

f32 = mybir.dt.float32
f32r = mybir.dt.float32r
i16 = mybir.dt.int16

_CACHE: dict = {}
LAST_RESULTS = None
LAST_SPMD = None


def _lpt_bins(length):
    """LPT-pack 2048 b's into NCORES*GRPS bins of exactly GB b's, minimizing
    max bin rows; then deal bins to cores (4 each) to equalize core totals,
    and swap-rebalance within each core toward per-rank row caps so the
    static tile total hits its lower bound (ceil(max_core_rows/128)).
    Returns asm[core, grp, slot] = b index, tiles[grp] = static tile count."""
    nbins = NCORES * GRPS
    order = np.argsort(-length, kind="stable")
    bin_rows = np.zeros(nbins, dtype=np.int64)
    bin_cnt = np.zeros(nbins, dtype=np.int64)
    bins = [[] for _ in range(nbins)]
    for b in order:
        k = np.argmin(bin_rows + (bin_cnt >= GB) * (1 << 40))
        bins[k].append(b)
        bin_rows[k] += length[b]
        bin_cnt[k] += 1
    assert (bin_cnt == GB).all()
    # deal bins to cores: sort bins desc by rows; serpentine over cores
    border = np.argsort(-bin_rows, kind="stable")
    asm = np.empty((NCORES, GRPS, GB), dtype=np.int64)
    core_grp_rows = np.zeros((NCORES, GRPS), dtype=np.int64)
    for r in range(GRPS):
        cores = range(NCORES) if r % 2 == 0 else range(NCORES - 1, -1, -1)
        for j, c in enumerate(cores):
            k = border[r * NCORES + j]
            asm[c, r] = np.array(bins[k], dtype=np.int64)
            core_grp_rows[c, r] = bin_rows[k]

    # Per-rank caps summing to the optimal tile count: distribute
    # ceil(max_core_rows/128) tiles over GRPS ranks (largest ranks first).
    worst = int(core_grp_rows.sum(axis=1).max())
    t_opt = -(-worst // 128)
    base = t_opt // GRPS
    caps_tiles = np.full(GRPS, base, dtype=np.int64)
    caps_tiles[: t_opt - base * GRPS] += 1
    caps = caps_tiles * 128
    # Greedy swap passes: for each core, push each over-cap rank under its
    # cap by swapping a long b out of it for a shorter b from a rank with
    # headroom. Falls back to the loose fit if swaps run out.
    for c in range(NCORES):
        for _ in range(200):
            over = [g for g in range(GRPS) if core_grp_rows[c, g] > caps[g]]
            if not over:
                break
            g = over[0]
            need = core_grp_rows[c, g] - caps[g]
            done = False
            for g2 in range(GRPS):
                if g2 == g:
                    continue
                room = caps[g2] - core_grp_rows[c, g2]
                if room <= 0:
                    continue
                # swap b_i (in g) for b_j (in g2): moves d = len_i - len_j
                # rows out of g; need 0 < d <= room. Prefer d >= need (one
                # swap suffices), then smallest such d; else largest d.
                la = length[asm[c, g]]
                lb = length[asm[c, g2]]
                dd = la[:, None] - lb[None, :]
                ok = (dd > 0) & (dd <= room)
                if not ok.any():
                    continue
                score = np.where(
                    ok, np.where(dd >= need, (1 << 20) - dd, dd), -1
                )
                i, j = np.unravel_index(int(np.argmax(score)), dd.shape)
                d = int(dd[i, j])
                asm[c, g][i], asm[c, g2][j] = asm[c, g2][j], asm[c, g][i]
                core_grp_rows[c, g] -= d
                core_grp_rows[c, g2] += d
                done = True
                break
            if not done:
                break

    tiles = np.maximum(1, (-(-core_grp_rows.max(axis=0) // 128))).astype(np.int64)
    return asm, tiles, core_grp_rows


def _call_plan(tiles):
    """Static per-call (group, slot_tiles, num_idxs, tile_base) plan shared by
    all cores. Returns list of dicts and total tile count."""
    plan = []
    tile_base = 0
    for g in range(GRPS):
        t = 0
        while t < tiles[g]:
            ct = int(min(CT, tiles[g] - t))
            plan.append(
                dict(grp=g, tiles=ct, ni=ct * 128, tile_base=tile_base + t,
                     tile_in_grp=t)
            )
            t += ct
        tile_base += int(tiles[g])
    return plan, int(tile_base)


def _build_bass(tiles, plan, t_total, ncalls):
    nc = bacc.Bacc("TRN2", detect_race_conditions=True)
    seq = nc.dram_tensor("seq", [BL, L, D], f32r, kind="ExternalInput")
    colw = nc.dram_tensor("colw", [128, t_total * 2], f32, kind="ExternalInput")
    idx_cols = t_total * 8  # 128 idx per tile / 16 = 8 int16 cols per tile
    gidx = nc.dram_tensor("gidx", [128, idx_cols], i16, kind="ExternalInput")
    outn = nc.dram_tensor("outn", [BL, D], f32, kind="ExternalOutput")

    rows = seq[:].rearrange("b l d -> (b l) d")  # [BL*L, D]

    # per-call tile->global-tile base and idx col offset
    grp_last_call = {}
    grp_first_call = {}
    for ci, cl in enumerate(plan):
        grp_last_call[cl["grp"]] = ci
        grp_first_call.setdefault(cl["grp"], ci)

    sem_names = [
        "isem", "csem", "prsem", "dsem0", "dsem1", "dsem2", "dsem3", "dvsem",
        "pesem", "osem", "fsem0", "fsem1", "iosem",
    ]
    sems = {n: nc.alloc_semaphore(n) for n in sem_names}
    isem, csem, prsem = sems["isem"], sems["csem"], sems["prsem"]
    iosem = sems["iosem"]
    dsem = [sems[f"dsem{i}"] for i in range(4)]
    dvsem, pesem = sems["dvsem"], sems["pesem"]
    osem = sems["osem"]
    fsem = [sems["fsem0"], sems["fsem1"]]

    # idx columns for the first 4 calls (loaded first so prep 0 can start
    # while the remaining constants stream in)
    first4_tiles = sum(cl["tiles"] for cl in plan[:4])
    first4_cols = first4_tiles * 8

    with (
        nc.Block() as block,
        nc.sbuf_tensor("idx_sb", [128, idx_cols], i16) as idx_sb,
        nc.sbuf_tensor("colw_sb", [128, t_total * 2], f32) as colw_sb,
        nc.sbuf_tensor("iota_sb", [128, GB], f32) as iota_sb,
        nc.sbuf_tensor("gbuf", [128, 4 * CT * D], f32r) as gbuf,
        nc.sbuf_tensor("selt", [128, t_total * GB], f32r) as selt,
        nc.sbuf_tensor("out_sb", [GB, 2 * D], f32) as out_sb,
        nc.psum_tensor("ps0", [GB, D], f32) as ps0,
        nc.psum_tensor("ps1", [GB, D], f32) as ps1,
    ):
        ps = [ps0, ps1]

        @block.sync
        def _(sync: bass.BassEngine):
            sync.dma_start(
                out=idx_sb[:, :first4_cols], in_=gidx[:, :first4_cols]
            ).then_inc(isem, 16)
            sync.dma_start(out=colw_sb[:], in_=colw[:]).then_inc(csem, 16)
            sync.dma_start(
                out=idx_sb[:, first4_cols:], in_=gidx[:, first4_cols:]
            ).then_inc(csem, 16)
            for g in range(GRPS):
                sync.wait_ge(osem, g + 1)
                if g >= 2:
                    # direct-wait so the race detector sees the fsem reuse
                    # ordered (implied transitively via osem anyway)
                    sync.wait_ge(fsem[g % 2], 16 * (g // 2))
                sync.dma_start(
                    out=outn[g * GB : (g + 1) * GB, :],
                    in_=out_sb[:, (g % 2) * D : (g % 2 + 1) * D],
                ).then_inc(fsem[g % 2], 16)
            sync.wait_ge(fsem[0], 32)
            sync.wait_ge(fsem[1], 32)

        @block.gpsimd
        def _(gp: bass.BassGpSimd):
            gp.iota(
                iota_sb[:],
                [[1, GB]],
                channel_multiplier=0,
                allow_small_or_imprecise_dtypes=True,
            ).then_inc(iosem, 1)
            gp.wait_ge(isem, 16)
            for ci, cl in enumerate(plan):
                slot = ci % 4
                if ci == 4:
                    gp.wait_ge(csem, 32)
                if ci >= 4:
                    gp.wait_ge(pesem, ci - 3)
                    # direct-wait: dsem reuse ordered for the race detector
                    # (implied transitively via pesem anyway)
                    gp.wait_ge(dsem[slot], 16 * (ci // 4))
                g = cl["grp"]
                gp.dma_gather(
                    gbuf[:, slot * CT * D : (slot * CT + cl["tiles"]) * D].rearrange(
                        "p (c e) -> p c e", e=D
                    ),
                    rows[g * GB * L : (g + 1) * GB * L, :],
                    idx_sb[:, cl["tile_base"] * 8 : (cl["tile_base"] + cl["tiles"]) * 8],
                    cl["ni"],
                    cl["ni"],
                    D,
                    prepare_only=True,
                    sem=dsem[slot],
                ).then_inc(prsem, 1)
                gp.wait_ge(prsem, ci + 1)
                gp.trigger_dma(count=1)

        @block.vector
        def _(dve: bass.BassEngine):
            dve.wait_ge(iosem, 1)
            dve.wait_ge(csem, 32)
            for t in range(t_total):
                dve.tensor_scalar(
                    out=selt[:, t * GB : (t + 1) * GB],
                    in0=iota_sb[:],
                    scalar1=colw_sb[:, 2 * t : 2 * t + 1],
                    scalar2=colw_sb[:, 2 * t + 1 : 2 * t + 2],
                    op0=mybir.AluOpType.is_equal,
                    op1=mybir.AluOpType.mult,
                ).then_inc(dvsem, 1)
            for g in range(GRPS):
                dve.wait_ge(pesem, grp_last_call[g] + 1)
                if g >= 2:
                    dve.wait_ge(fsem[g % 2], 16 * (g // 2))
                dve.tensor_copy(
                    out=out_sb[:, (g % 2) * D : (g % 2 + 1) * D], in_=ps[g % 2][:]
                ).then_inc(osem, 1)

        @block.tensor
        def _(pe: bass.BassEngine):
            for ci, cl in enumerate(plan):
                slot = ci % 4
                g = cl["grp"]
                if ci == grp_first_call[g] and g >= 2:
                    pe.wait_ge(osem, g - 1)
                pe.wait_ge(dsem[slot], 16 * (ci // 4 + 1))
                pe.wait_ge(dvsem, cl["tile_base"] + cl["tiles"])
                for t in range(cl["tiles"]):
                    tt = cl["tile_base"] + t
                    mm = pe.matmul(
                        out=ps[g % 2][:],
                        lhsT=selt[:, tt * GB : (tt + 1) * GB],
                        rhs=gbuf[:, (slot * CT + t) * D : (slot * CT + t + 1) * D],
                        start=(cl["tile_in_grp"] + t == 0),
                        stop=(cl["tile_in_grp"] + t == int(tiles[g]) - 1),
                    )
                mm.then_inc(pesem, 1)

    nc.clear_and_free_semaphores(list(sems.values()))
    nc.compile()
    return nc


def _host_prep(begin_g, end_g, tiles, plan, t_total):
    """Per-core static gather indices + per-tile (colidx, w) columns.

    begin_g/end_g: [GRPS, GB] per (group, slot). Padding lanes gather row 0
    of the group region with colidx=-1 (weight 0)."""
    idx_all = np.zeros((t_total * 128,), dtype=np.int64)
    colidx = np.full((t_total, 128), -1.0, dtype=np.float32)
    wcol = np.zeros((t_total, 128), dtype=np.float32)
    tile_base = 0
    for g in range(GRPS):
        lens = (end_g[g] - begin_g[g]).astype(np.int64)
        n_rows = int(lens.sum())
        assert n_rows <= int(tiles[g]) * 128
        slots = np.repeat(np.arange(GB), lens)
        ls = np.concatenate(
            [np.arange(begin_g[g][s], end_g[g][s]) for s in range(GB)]
        )
        ridx = slots * L + ls
        base = tile_base * 128
        idx_all[base : base + n_rows] = ridx
        tpos = np.arange(n_rows)
        colidx[tile_base + tpos // 128, tpos % 128] = slots.astype(np.float32)
        wcol[tile_base + tpos // 128, tpos % 128] = (1.0 / lens[slots]).astype(
            np.float32
        )
        tile_base += int(tiles[g])
    assert idx_all.max() < GB * L
    idx16 = idx_all.astype(np.int16).reshape(-1, 16).T  # [16, total/16]
    idx = np.ascontiguousarray(np.tile(idx16, (8, 1)))  # [128, total/16]
    colw = np.empty((128, t_total * 2), dtype=np.float32)
    colw[:, 0::2] = colidx.T
    colw[:, 1::2] = wcol.T
    return np.ascontiguousarray(colw), idx


def _prep_all(begin_i, end_i):
    length = end_i - begin_i
    asm, tiles, _ = _lpt_bins(length)
    plan, t_total = _call_plan(tiles)
    return asm, tiles, plan, t_total


def kernel(seq, begin, end):
    global LAST_RESULTS, LAST_SPMD
    seq = np.ascontiguousarray(np.asarray(seq, dtype=np.float32))
    begin_i = np.asarray(begin).astype(np.int64)
    end_i = np.asarray(end).astype(np.int64)
    asm, tiles, plan, t_total = _prep_all(begin_i, end_i)

    key = (tuple(int(t) for t in tiles),)
    if key not in _CACHE:
        _CACHE.clear()
        _CACHE[key] = _build_bass(tiles, plan, t_total, len(plan))
    nc = _CACHE[key]

    in_maps = []
    for c in range(NCORES):
        colw, idx = _host_prep(
            begin_i[asm[c]], end_i[asm[c]], tiles, plan, t_total
        )
        in_maps.append(
            {"seq": seq[asm[c].reshape(-1)], "colw": colw, "gidx": idx}
        )

    LAST_SPMD = (nc, in_maps)
    last_exc = None
    for attempt in range(3):
        try:
            LAST_RESULTS = run_bass_kernel_spmd(
                nc, in_maps, core_ids=list(range(NCORES))
            )
            break
        except Exception as e:  # noqa: BLE001
            last_exc = e
            time.sleep(10.0)
    else:
        raise last_exc
    out = np.empty((B, D), dtype=np.float32)
    for c in range(NCORES):
        out[asm[c].reshape(-1)] = LAST_RESULTS.results[c]["outn"]
    return out


# revision 3
# speedup vs baseline: 1.0008x; 1.0008x over previous
"""V7: fully-static raw-bass ragged segment mean, prep/trigger pipelined gather.

Strategy (vs V6 baseline):
- Compile happens inside kernel() AFTER seeing begin/end, so the whole
  program is specialized to the actual ragged sizes: LPT-balanced core/group
  assignment, exact static tile/call counts, static num_idxs — no
  value_load, no empty boot-guard calls, no runtime-count path.
- Gathers use dma_gather(prepare_only=True) + trigger_dma so Q7 descriptor
  generation overlaps the SDMA transfers; transfers run back-to-back at the
  HBM-per-core roofline (~360 GB/s). The non-prep path serializes desc-gen
  (994ns + 0.34ns/desc) with each 2MB transfer.
- Raw bass (no TileContext): manual semaphore choreography, engines:
    SP   : const loads (idx/colw/iota), per-group output stores
    Pool : per call: prep (sem=dsem[slot]) -> wait prep-sem -> trigger
    DVE  : all per-tile selection matrices up front, then psum->SBUF drains
    PE   : per tile: psum[group] += sel[128,64].T @ gtile[128,512] (f32r)
- Padding rows gather row 0 of the group region with weight 0 (no -1
  indices: every engine always gets descriptors so completion sems fire,
  and sim traffic == HW traffic).

Per-core bytes ~= 33.9k rows * 2KiB ~= 69.5 MB -> ~195 us transfer floor.
"""

import time

import numpy as np

import concourse.bass as bass
from concourse import bacc
import concourse.mybir as mybir
from concourse.bass_utils import run_bass_kernel_spmd

B, L, D = 2048, 512, 512
NCORES = 8
BL = B // NCORES  # 256
GB = 64  # b's per group (region = GB*L = 32768 rows -> int16 idx max)
GRPS = BL // GB  # 4
CT = 8  # max tiles per dma_gather call (8*128=1024 idx)

f32 = mybir.dt.float32
f32r = mybir.dt.float32r
i16 = mybir.dt.int16

_CACHE: dict = {}
LAST_RESULTS = None
LAST_SPMD = None


def _lpt_bins(length):
    """LPT-pack 2048 b's into NCORES*GRPS bins of exactly GB b's, minimizing
    max bin rows; then deal bins to cores (4 each) to equalize core totals,
    and swap-rebalance within each core toward per-rank row caps so the
    static tile total hits its lower bound (ceil(max_core_rows/128)).
    Returns asm[core, grp, slot] = b index, tiles[grp] = static tile count."""
    nbins = NCORES * GRPS
    order = np.argsort(-length, kind="stable")
    bin_rows = np.zeros(nbins, dtype=np.int64)
    bin_cnt = np.zeros(nbins, dtype=np.int64)
    bins = [[] for _ in range(nbins)]
    for b in order:
        k = np.argmin(bin_rows + (bin_cnt >= GB) * (1 << 40))
        bins[k].append(b)
        bin_rows[k] += length[b]
        bin_cnt[k] += 1
    assert (bin_cnt == GB).all()
    # deal bins to cores: sort bins desc by rows; serpentine over cores
    border = np.argsort(-bin_rows, kind="stable")
    asm = np.empty((NCORES, GRPS, GB), dtype=np.int64)
    core_grp_rows = np.zeros((NCORES, GRPS), dtype=np.int64)
    for r in range(GRPS):
        cores = range(NCORES) if r % 2 == 0 else range(NCORES - 1, -1, -1)
        for j, c in enumerate(cores):
            k = border[r * NCORES + j]
            asm[c, r] = np.array(bins[k], dtype=np.int64)
            core_grp_rows[c, r] = bin_rows[k]

    # Per-rank caps summing to the optimal tile count: distribute
    # ceil(max_core_rows/128) tiles over GRPS ranks (largest ranks first).
    worst = int(core_grp_rows.sum(axis=1).max())
    t_opt = -(-worst // 128)
    base = t_opt // GRPS
    caps_tiles = np.full(GRPS, base, dtype=np.int64)
    caps_tiles[: t_opt - base * GRPS] += 1
    caps = caps_tiles * 128
    # Greedy swap passes: for each core, push each over-cap rank under its
    # cap by swapping a long b out of it for a shorter b from a rank with
    # headroom. Falls back to the loose fit if swaps run out.
    for c in range(NCORES):
        for _ in range(200):
            over = [g for g in range(GRPS) if core_grp_rows[c, g] > caps[g]]
            if not over:
                break
            g = over[0]
            need = core_grp_rows[c, g] - caps[g]
            done = False
            for g2 in range(GRPS):
                if g2 == g:
                    continue
                room = caps[g2] - core_grp_rows[c, g2]
                if room <= 0:
                    continue
                # swap b_i (in g) for b_j (in g2): moves d = len_i - len_j
                # rows out of g; need 0 < d <= room. Prefer d >= need (one
                # swap suffices), then smallest such d; else largest d.
                la = length[asm[c, g]]
                lb = length[asm[c, g2]]
                dd = la[:, None] - lb[None, :]
                ok = (dd > 0) & (dd <= room)
                if not ok.any():
                    continue
                score = np.where(
                    ok, np.where(dd >= need, (1 << 20) - dd, dd), -1
                )
                i, j = np.unravel_index(int(np.argmax(score)), dd.shape)
                d = int(dd[i, j])
                asm[c, g][i], asm[c, g2][j] = asm[c, g2][j], asm[c, g][i]
                core_grp_rows[c, g] -= d
                core_grp_rows[c, g2] += d
                done = True
                break
            if not done:
                break

    tiles = np.maximum(1, (-(-core_grp_rows.max(axis=0) // 128))).astype(np.int64)
    return asm, tiles, core_grp_rows


def _call_plan(tiles):
    """Static per-call (group, slot_tiles, num_idxs, tile_base) plan shared by
    all cores. Returns list of dicts and total tile count."""
    plan = []
    tile_base = 0
    for g in range(GRPS):
        t = 0
        while t < tiles[g]:
            ct = int(min(CT, tiles[g] - t))
            plan.append(
                dict(grp=g, tiles=ct, ni=ct * 128, tile_base=tile_base + t,
                     tile_in_grp=t)
            )
            t += ct
        tile_base += int(tiles[g])
    return plan, int(tile_base)


def _build_bass(tiles, plan, t_total, ncalls):
    nc = bacc.Bacc("TRN2", detect_race_conditions=True)
    seq = nc.dram_tensor("seq", [BL, L, D], f32r, kind="ExternalInput")
    colw = nc.dram_tensor("colw", [128, t_total * 2], f32, kind="ExternalInput")
    idx_cols = t_total * 8  # 128 idx per tile / 16 = 8 int16 cols per tile
    gidx = nc.dram_tensor("gidx", [128, idx_cols], i16, kind="ExternalInput")
    outn = nc.dram_tensor("outn", [BL, D], f32, kind="ExternalOutput")

    rows = seq[:].rearrange("b l d -> (b l) d")  # [BL*L, D]

    # per-call tile->global-tile base and idx col offset
    grp_last_call = {}
    grp_first_call = {}
    for ci, cl in enumerate(plan):
        grp_last_call[cl["grp"]] = ci
        grp_first_call.setdefault(cl["grp"], ci)

    sem_names = [
        "isem", "csem", "prsem", "dsem0", "dsem1", "dsem2", "dsem3", "dvsem",
        "pesem", "osem", "fsem0", "fsem1", "iosem",
    ]
    sems = {n: nc.alloc_semaphore(n) for n in sem_names}
    isem, csem, prsem = sems["isem"], sems["csem"], sems["prsem"]
    iosem = sems["iosem"]
    dsem = [sems[f"dsem{i}"] for i in range(4)]
    dvsem, pesem = sems["dvsem"], sems["pesem"]
    osem = sems["osem"]
    fsem = [sems["fsem0"], sems["fsem1"]]

    # idx columns for the first 4 calls (loaded first so prep 0 can start
    # while the remaining constants stream in)
    first4_tiles = sum(cl["tiles"] for cl in plan[:4])
    first4_cols = first4_tiles * 8

    with (
        nc.Block() as block,
        nc.sbuf_tensor("idx_sb", [128, idx_cols], i16) as idx_sb,
        nc.sbuf_tensor("colw_sb", [128, t_total * 2], f32) as colw_sb,
        nc.sbuf_tensor("iota_sb", [128, GB], f32) as iota_sb,
        nc.sbuf_tensor("gbuf", [128, 4 * CT * D], f32r) as gbuf,
        nc.sbuf_tensor("selt", [128, t_total * GB], f32r) as selt,
        nc.sbuf_tensor("out_sb", [GB, 2 * D], f32) as out_sb,
        nc.psum_tensor("ps0", [GB, D], f32) as ps0,
        nc.psum_tensor("ps1", [GB, D], f32) as ps1,
    ):
        ps = [ps0, ps1]

        @block.sync
        def _(sync: bass.BassEngine):
            sync.dma_start(
                out=idx_sb[:, :first4_cols], in_=gidx[:, :first4_cols]
            ).then_inc(isem, 16)
            sync.dma_start(out=colw_sb[:], in_=colw[:]).then_inc(csem, 16)
            sync.dma_start(
                out=idx_sb[:, first4_cols:], in_=gidx[:, first4_cols:]
            ).then_inc(csem, 16)
            for g in range(GRPS):
                sync.wait_ge(osem, g + 1)
                if g >= 2:
                    # direct-wait so the race detector sees the fsem reuse
                    # ordered (implied transitively via osem anyway)
                    sync.wait_ge(fsem[g % 2], 16 * (g // 2))
                sync.dma_start(
                    out=outn[g * GB : (g + 1) * GB, :],
                    in_=out_sb[:, (g % 2) * D : (g % 2 + 1) * D],
                ).then_inc(fsem[g % 2], 16)
            sync.wait_ge(fsem[0], 32)
            sync.wait_ge(fsem[1], 32)

        @block.gpsimd
        def _(gp: bass.BassGpSimd):
            gp.iota(
                iota_sb[:],
                [[1, GB]],
                channel_multiplier=0,
                allow_small_or_imprecise_dtypes=True,
            ).then_inc(iosem, 1)
            gp.wait_ge(isem, 16)
            for ci, cl in enumerate(plan):
                slot = ci % 4
                if ci == 4:
                    gp.wait_ge(csem, 32)
                if ci >= 4:
                    gp.wait_ge(pesem, ci - 3)
                    # direct-wait: dsem reuse ordered for the race detector
                    # (implied transitively via pesem anyway)
                    gp.wait_ge(dsem[slot], 16 * (ci // 4))
                g = cl["grp"]
                gp.dma_gather(
                    gbuf[:, slot * CT * D : (slot * CT + cl["tiles"]) * D].rearrange(
                        "p (c e) -> p c e", e=D
                    ),
                    rows[g * GB * L : (g + 1) * GB * L, :],
                    idx_sb[:, cl["tile_base"] * 8 : (cl["tile_base"] + cl["tiles"]) * 8],
                    cl["ni"],
                    cl["ni"],
                    D,
                    prepare_only=True,
                    sem=dsem[slot],
                ).then_inc(prsem, 1)
                gp.wait_ge(prsem, ci + 1)
                gp.trigger_dma(count=1)

        @block.vector
        def _(dve: bass.BassEngine):
            dve.wait_ge(iosem, 1)
            dve.wait_ge(csem, 32)
            for t in range(t_total):
                dve.tensor_scalar(
                    out=selt[:, t * GB : (t + 1) * GB],
                    in0=iota_sb[:],
                    scalar1=colw_sb[:, 2 * t : 2 * t + 1],
                    scalar2=colw_sb[:, 2 * t + 1 : 2 * t + 2],
                    op0=mybir.AluOpType.is_equal,
                    op1=mybir.AluOpType.mult,
                ).then_inc(dvsem, 1)
            for g in range(GRPS):
                dve.wait_ge(pesem, grp_last_call[g] + 1)
                if g >= 2:
                    dve.wait_ge(fsem[g % 2], 16 * (g // 2))
                dve.tensor_copy(
                    out=out_sb[:, (g % 2) * D : (g % 2 + 1) * D], in_=ps[g % 2][:]
                ).then_inc(osem, 1)

        @block.tensor
        def _(pe: bass.BassEngine):
            for ci, cl in enumerate(plan):
                slot = ci % 4
                g = cl["grp"]
                if ci == grp_first_call[g] and g >= 2:
                    pe.wait_ge(osem, g - 1)
                pe.wait_ge(dsem[slot], 16 * (ci // 4 + 1))
                pe.wait_ge(dvsem, cl["tile_base"] + cl["tiles"])
                for t in range(cl["tiles"]):
                    tt = cl["tile_base"] + t
                    mm = pe.matmul(
                        out=ps[g % 2][:],
                        lhsT=selt[:, tt * GB : (tt + 1) * GB],
                        rhs=gbuf[:, (slot * CT + t) * D : (slot * CT + t + 1) * D],
                        start=(cl["tile_in_grp"] + t == 0),
                        stop=(cl["tile_in_grp"] + t == int(tiles[g]) - 1),
                    )
                mm.then_inc(pesem, 1)

    nc.clear_and_free_semaphores(list(sems.values()))
    nc.compile()
    return nc


def _host_prep(begin_g, end_g, tiles, plan, t_total):
    """Per-core static gather indices + per-tile (colidx, w) columns.

    begin_g/end_g: [GRPS, GB] per (group, slot). Padding lanes gather row 0
    of the group region with colidx=-1 (weight 0)."""
    idx_all = np.zeros((t_total * 128,), dtype=np.int64)
    colidx = np.full((t_total, 128), -1.0, dtype=np.float32)
    wcol = np.zeros((t_total, 128), dtype=np.float32)
    tile_base = 0
    for g in range(GRPS):
        lens = (end_g[g] - begin_g[g]).astype(np.int64)
        n_rows = int(lens.sum())
        assert n_rows <= int(tiles[g]) * 128
        slots = np.repeat(np.arange(GB), lens)
        ls = np.concatenate(
            [np.arange(begin_g[g][s], end_g[g][s]) for s in range(GB)]
        )
        ridx = slots * L + ls
        base = tile_base * 128
        idx_all[base : base + n_rows] = ridx
        tpos = np.arange(n_rows)
        colidx[tile_base + tpos // 128, tpos % 128] = slots.astype(np.float32)
        wcol[tile_base + tpos // 128, tpos % 128] = (1.0 / lens[slots]).astype(
            np.float32
        )
        tile_base += int(tiles[g])
    assert idx_all.max() < GB * L
    idx16 = idx_all.astype(np.int16).reshape(-1, 16).T  # [16, total/16]
    idx = np.ascontiguousarray(np.tile(idx16, (8, 1)))  # [128, total/16]
    colw = np.empty((128, t_total * 2), dtype=np.float32)
    colw[:, 0::2] = colidx.T
    colw[:, 1::2] = wcol.T
    return np.ascontiguousarray(colw), idx


def _prep_all(begin_i, end_i):
    length = end_i - begin_i
    asm, tiles, _ = _lpt_bins(length)
    plan, t_total = _call_plan(tiles)
    return asm, tiles, plan, t_total


def kernel(seq, begin, end):
    global LAST_RESULTS, LAST_SPMD
    seq = np.ascontiguousarray(np.asarray(seq, dtype=np.float32))
    begin_i = np.asarray(begin).astype(np.int64)
    end_i = np.asarray(end).astype(np.int64)
    asm, tiles, plan, t_total = _prep_all(begin_i, end_i)

    key = (tuple(int(t) for t in tiles),)
    if key not in _CACHE:
        _CACHE.clear()
        _CACHE[key] = _build_bass(tiles, plan, t_total, len(plan))
    nc = _CACHE[key]

    in_maps = []
    for c in range(NCORES):
        colw, idx = _host_prep(
            begin_i[asm[c]], end_i[asm[c]], tiles, plan, t_total
        )
        in_maps.append(
            {"seq": seq[asm[c].reshape(-1)], "colw": colw, "gidx": idx}
        )

    LAST_SPMD = (nc, in_maps)
    last_exc = None
    out = np.empty((B, D), dtype=np.float32)
    for attempt in range(3):
        try:
            LAST_RESULTS = run_bass_kernel_spmd(
                nc, in_maps, core_ids=list(range(NCORES))
            )
            # materialize inside the retry: transient axon exec errors can
            # surface only at device->host fetch time
            for c in range(NCORES):
                out[asm[c].reshape(-1)] = np.asarray(
                    LAST_RESULTS.results[c]["outn"]
                )
            break
        except Exception as e:  # noqa: BLE001
            last_exc = e
            time.sleep(10.0)
    else:
        raise last_exc
    return out


# revision 4
# speedup vs baseline: 1.0068x; 1.0060x over previous
"""V7: fully-static raw-bass ragged segment mean, prep/trigger pipelined gather.

Strategy (vs V6 baseline):
- Compile happens inside kernel() AFTER seeing begin/end, so the whole
  program is specialized to the actual ragged sizes: LPT-balanced core/group
  assignment, exact static tile/call counts, static num_idxs — no
  value_load, no empty boot-guard calls, no runtime-count path.
- Gathers use dma_gather(prepare_only=True) + trigger_dma so Q7 descriptor
  generation overlaps the SDMA transfers; transfers run back-to-back at the
  HBM-per-core roofline (~360 GB/s). The non-prep path serializes desc-gen
  (994ns + 0.34ns/desc) with each 2MB transfer.
- Raw bass (no TileContext): manual semaphore choreography, engines:
    SP   : const loads (idx/colw/iota), per-group output stores
    Pool : per call: prep (sem=dsem[slot]) -> wait prep-sem -> trigger
    DVE  : all per-tile selection matrices up front, then psum->SBUF drains
    PE   : per tile: psum[group] += sel[128,64].T @ gtile[128,512] (f32r)
- Padding rows gather row 0 of the group region with weight 0 (no -1
  indices: every engine always gets descriptors so completion sems fire,
  and sim traffic == HW traffic).

- The last group's final ~10 tiles go in 2-tile calls so the tail matmuls
  pipeline with the small transfers (dsem is per-call; a trailing 8-tile
  call strands 8 matmuls behind the last transfer).
- Selection matrices are pure equality masks; the 1/len scaling happens
  once per group in the psum->SBUF drain (tensor_scalar mult with a
  per-slot scalar column) — halves the colw constant and matches the
  reference's sum-then-divide numerics.

Per-core bytes = 33,920 rows * 2KiB = 69.5 MB -> 193.0 us transfer floor at
the 360 GB/s HBM-per-NC limit; TimelineSim of this exact program: 204.9 us.
"""

import time

import numpy as np

import concourse.bass as bass
from concourse import bacc
import concourse.mybir as mybir
from concourse.bass_utils import run_bass_kernel_spmd

B, L, D = 2048, 512, 512
NCORES = 8
BL = B // NCORES  # 256
GB = 64  # b's per group (region = GB*L = 32768 rows -> int16 idx max)
GRPS = BL // GB  # 4
CT = 8  # max tiles per dma_gather call (8*128=1024 idx)

f32 = mybir.dt.float32
f32r = mybir.dt.float32r
i16 = mybir.dt.int16

_CACHE: dict = {}
LAST_RESULTS = None
LAST_SPMD = None


def _lpt_bins(length):
    """LPT-pack 2048 b's into NCORES*GRPS bins of exactly GB b's, minimizing
    max bin rows; then deal bins to cores (4 each) to equalize core totals,
    and swap-rebalance within each core toward per-rank row caps so the
    static tile total hits its lower bound (ceil(max_core_rows/128)).
    Returns asm[core, grp, slot] = b index, tiles[grp] = static tile count."""
    nbins = NCORES * GRPS
    order = np.argsort(-length, kind="stable")
    bin_rows = np.zeros(nbins, dtype=np.int64)
    bin_cnt = np.zeros(nbins, dtype=np.int64)
    bins = [[] for _ in range(nbins)]
    for b in order:
        k = np.argmin(bin_rows + (bin_cnt >= GB) * (1 << 40))
        bins[k].append(b)
        bin_rows[k] += length[b]
        bin_cnt[k] += 1
    assert (bin_cnt == GB).all()
    # deal bins to cores: sort bins desc by rows; serpentine over cores
    border = np.argsort(-bin_rows, kind="stable")
    asm = np.empty((NCORES, GRPS, GB), dtype=np.int64)
    core_grp_rows = np.zeros((NCORES, GRPS), dtype=np.int64)
    for r in range(GRPS):
        cores = range(NCORES) if r % 2 == 0 else range(NCORES - 1, -1, -1)
        for j, c in enumerate(cores):
            k = border[r * NCORES + j]
            asm[c, r] = np.array(bins[k], dtype=np.int64)
            core_grp_rows[c, r] = bin_rows[k]

    # Per-rank caps summing to the optimal tile count: distribute
    # ceil(max_core_rows/128) tiles over GRPS ranks (largest ranks first).
    worst = int(core_grp_rows.sum(axis=1).max())
    t_opt = -(-worst // 128)
    base = t_opt // GRPS
    caps_tiles = np.full(GRPS, base, dtype=np.int64)
    caps_tiles[: t_opt - base * GRPS] += 1
    caps = caps_tiles * 128
    # Greedy swap passes: for each core, push each over-cap rank under its
    # cap by swapping a long b out of it for a shorter b from a rank with
    # headroom. Falls back to the loose fit if swaps run out.
    for c in range(NCORES):
        for _ in range(200):
            over = [g for g in range(GRPS) if core_grp_rows[c, g] > caps[g]]
            if not over:
                break
            g = over[0]
            need = core_grp_rows[c, g] - caps[g]
            done = False
            for g2 in range(GRPS):
                if g2 == g:
                    continue
                room = caps[g2] - core_grp_rows[c, g2]
                if room <= 0:
                    continue
                # swap b_i (in g) for b_j (in g2): moves d = len_i - len_j
                # rows out of g; need 0 < d <= room. Prefer d >= need (one
                # swap suffices), then smallest such d; else largest d.
                la = length[asm[c, g]]
                lb = length[asm[c, g2]]
                dd = la[:, None] - lb[None, :]
                ok = (dd > 0) & (dd <= room)
                if not ok.any():
                    continue
                score = np.where(
                    ok, np.where(dd >= need, (1 << 20) - dd, dd), -1
                )
                i, j = np.unravel_index(int(np.argmax(score)), dd.shape)
                d = int(dd[i, j])
                asm[c, g][i], asm[c, g2][j] = asm[c, g2][j], asm[c, g][i]
                core_grp_rows[c, g] -= d
                core_grp_rows[c, g2] += d
                done = True
                break
            if not done:
                break

    tiles = np.maximum(1, (-(-core_grp_rows.max(axis=0) // 128))).astype(np.int64)
    return asm, tiles, core_grp_rows


def _call_plan(tiles):
    """Static per-call (group, slot_tiles, num_idxs, tile_base) plan shared by
    all cores. Returns list of dicts and total tile count."""
    plan = []
    tile_base = 0
    for g in range(GRPS):
        taper = 10 if g == GRPS - 1 else 0
        t = 0
        while t < tiles[g]:
            left = int(tiles[g]) - t
            ct = min(CT, left) if left > taper else min(2, left)
            plan.append(
                dict(grp=g, tiles=ct, ni=ct * 128, tile_base=tile_base + t,
                     tile_in_grp=t)
            )
            t += ct
        tile_base += int(tiles[g])
    return plan, int(tile_base)


def _build_bass(tiles, plan, t_total, ncalls):
    nc = bacc.Bacc("TRN2", detect_race_conditions=True)
    seq = nc.dram_tensor("seq", [BL, L, D], f32r, kind="ExternalInput")
    colw = nc.dram_tensor("colw", [128, t_total], f32, kind="ExternalInput")
    wts = nc.dram_tensor("wts", [GB, GRPS], f32, kind="ExternalInput")
    idx_cols = t_total * 8  # 128 idx per tile / 16 = 8 int16 cols per tile
    gidx = nc.dram_tensor("gidx", [128, idx_cols], i16, kind="ExternalInput")
    outn = nc.dram_tensor("outn", [BL, D], f32, kind="ExternalOutput")

    rows = seq[:].rearrange("b l d -> (b l) d")  # [BL*L, D]

    # per-call tile->global-tile base and idx col offset
    grp_last_call = {}
    grp_first_call = {}
    for ci, cl in enumerate(plan):
        grp_last_call[cl["grp"]] = ci
        grp_first_call.setdefault(cl["grp"], ci)

    sem_names = [
        "isem", "csem", "prsem", "dsem0", "dsem1", "dsem2", "dsem3", "dvsem",
        "pesem", "osem", "fsem0", "fsem1", "iosem",
    ]
    sems = {n: nc.alloc_semaphore(n) for n in sem_names}
    isem, csem, prsem = sems["isem"], sems["csem"], sems["prsem"]
    iosem = sems["iosem"]
    dsem = [sems[f"dsem{i}"] for i in range(4)]
    dvsem, pesem = sems["dvsem"], sems["pesem"]
    osem = sems["osem"]
    fsem = [sems["fsem0"], sems["fsem1"]]

    # idx columns for the first 4 calls (loaded first so prep 0 can start
    # while the remaining constants stream in)
    first4_tiles = sum(cl["tiles"] for cl in plan[:4])
    first4_cols = first4_tiles * 8

    with (
        nc.Block() as block,
        nc.sbuf_tensor("idx_sb", [128, idx_cols], i16) as idx_sb,
        nc.sbuf_tensor("colw_sb", [128, t_total], f32) as colw_sb,
        nc.sbuf_tensor("wts_sb", [GB, GRPS], f32) as wts_sb,
        nc.sbuf_tensor("iota_sb", [128, GB], f32) as iota_sb,
        nc.sbuf_tensor("gbuf", [128, 4 * CT * D], f32r) as gbuf,
        nc.sbuf_tensor("selt", [128, t_total * GB], f32r) as selt,
        nc.sbuf_tensor("out_sb", [GB, 2 * D], f32) as out_sb,
        nc.psum_tensor("ps0", [GB, D], f32) as ps0,
        nc.psum_tensor("ps1", [GB, D], f32) as ps1,
    ):
        ps = [ps0, ps1]

        @block.sync
        def _(sync: bass.BassEngine):
            sync.dma_start(
                out=idx_sb[:, :first4_cols], in_=gidx[:, :first4_cols]
            ).then_inc(isem, 16)
            sync.dma_start(out=colw_sb[:], in_=colw[:]).then_inc(csem, 16)
            sync.dma_start(
                out=idx_sb[:, first4_cols:], in_=gidx[:, first4_cols:]
            ).then_inc(csem, 16)
            sync.dma_start(out=wts_sb[:], in_=wts[:]).then_inc(csem, 16)
            for g in range(GRPS):
                sync.wait_ge(osem, g + 1)
                if g >= 2:
                    # direct-wait so the race detector sees the fsem reuse
                    # ordered (implied transitively via osem anyway)
                    sync.wait_ge(fsem[g % 2], 16 * (g // 2))
                sync.dma_start(
                    out=outn[g * GB : (g + 1) * GB, :],
                    in_=out_sb[:, (g % 2) * D : (g % 2 + 1) * D],
                ).then_inc(fsem[g % 2], 16)
            sync.wait_ge(fsem[0], 32)
            sync.wait_ge(fsem[1], 32)

        @block.gpsimd
        def _(gp: bass.BassGpSimd):
            gp.iota(
                iota_sb[:],
                [[1, GB]],
                channel_multiplier=0,
                allow_small_or_imprecise_dtypes=True,
            ).then_inc(iosem, 1)
            gp.wait_ge(isem, 16)
            for ci, cl in enumerate(plan):
                slot = ci % 4
                if ci == 4:
                    gp.wait_ge(csem, 48)
                if ci >= 4:
                    gp.wait_ge(pesem, ci - 3)
                    # direct-wait: dsem reuse ordered for the race detector
                    # (implied transitively via pesem anyway)
                    gp.wait_ge(dsem[slot], 16 * (ci // 4))
                g = cl["grp"]
                gp.dma_gather(
                    gbuf[:, slot * CT * D : (slot * CT + cl["tiles"]) * D].rearrange(
                        "p (c e) -> p c e", e=D
                    ),
                    rows[g * GB * L : (g + 1) * GB * L, :],
                    idx_sb[:, cl["tile_base"] * 8 : (cl["tile_base"] + cl["tiles"]) * 8],
                    cl["ni"],
                    cl["ni"],
                    D,
                    prepare_only=True,
                    sem=dsem[slot],
                ).then_inc(prsem, 1)
                gp.wait_ge(prsem, ci + 1)
                gp.trigger_dma(count=1)

        @block.vector
        def _(dve: bass.BassEngine):
            dve.wait_ge(iosem, 1)
            dve.wait_ge(csem, 48)
            for t in range(t_total):
                dve.tensor_scalar(
                    out=selt[:, t * GB : (t + 1) * GB],
                    in0=iota_sb[:],
                    scalar1=colw_sb[:, t : t + 1],
                    scalar2=None,
                    op0=mybir.AluOpType.is_equal,
                ).then_inc(dvsem, 1)
            for g in range(GRPS):
                dve.wait_ge(pesem, grp_last_call[g] + 1)
                if g >= 2:
                    dve.wait_ge(fsem[g % 2], 16 * (g // 2))
                dve.tensor_scalar(
                    out=out_sb[:, (g % 2) * D : (g % 2 + 1) * D],
                    in0=ps[g % 2][:],
                    scalar1=wts_sb[:, g : g + 1],
                    scalar2=None,
                    op0=mybir.AluOpType.mult,
                ).then_inc(osem, 1)

        @block.tensor
        def _(pe: bass.BassEngine):
            for ci, cl in enumerate(plan):
                slot = ci % 4
                g = cl["grp"]
                if ci == grp_first_call[g] and g >= 2:
                    pe.wait_ge(osem, g - 1)
                pe.wait_ge(dsem[slot], 16 * (ci // 4 + 1))
                pe.wait_ge(dvsem, cl["tile_base"] + cl["tiles"])
                for t in range(cl["tiles"]):
                    tt = cl["tile_base"] + t
                    mm = pe.matmul(
                        out=ps[g % 2][:],
                        lhsT=selt[:, tt * GB : (tt + 1) * GB],
                        rhs=gbuf[:, (slot * CT + t) * D : (slot * CT + t + 1) * D],
                        start=(cl["tile_in_grp"] + t == 0),
                        stop=(cl["tile_in_grp"] + t == int(tiles[g]) - 1),
                    )
                mm.then_inc(pesem, 1)

    nc.clear_and_free_semaphores(list(sems.values()))
    nc.compile()
    return nc


def _host_prep(begin_g, end_g, tiles, plan, t_total):
    """Per-core static gather indices + per-tile (colidx, w) columns.

    begin_g/end_g: [GRPS, GB] per (group, slot). Padding lanes gather row 0
    of the group region with colidx=-1 (weight 0)."""
    idx_all = np.zeros((t_total * 128,), dtype=np.int64)
    colidx = np.full((t_total, 128), -1.0, dtype=np.float32)
    tile_base = 0
    for g in range(GRPS):
        lens = (end_g[g] - begin_g[g]).astype(np.int64)
        n_rows = int(lens.sum())
        assert n_rows <= int(tiles[g]) * 128
        slots = np.repeat(np.arange(GB), lens)
        ls = np.concatenate(
            [np.arange(begin_g[g][s], end_g[g][s]) for s in range(GB)]
        )
        ridx = slots * L + ls
        base = tile_base * 128
        idx_all[base : base + n_rows] = ridx
        tpos = np.arange(n_rows)
        colidx[tile_base + tpos // 128, tpos % 128] = slots.astype(np.float32)
        tile_base += int(tiles[g])
    assert idx_all.max() < GB * L
    idx16 = idx_all.astype(np.int16).reshape(-1, 16).T  # [16, total/16]
    idx = np.ascontiguousarray(np.tile(idx16, (8, 1)))  # [128, total/16]
    colw = np.ascontiguousarray(colidx.T)  # [128, t_total]
    wts = np.zeros((GB, GRPS), dtype=np.float32)
    for g in range(GRPS):
        wts[:, g] = 1.0 / (end_g[g] - begin_g[g]).astype(np.float32)
    return colw, idx, wts


def _prep_all(begin_i, end_i):
    length = end_i - begin_i
    asm, tiles, _ = _lpt_bins(length)
    plan, t_total = _call_plan(tiles)
    return asm, tiles, plan, t_total


def kernel(seq, begin, end):
    global LAST_RESULTS, LAST_SPMD
    seq = np.ascontiguousarray(np.asarray(seq, dtype=np.float32))
    begin_i = np.asarray(begin).astype(np.int64)
    end_i = np.asarray(end).astype(np.int64)
    asm, tiles, plan, t_total = _prep_all(begin_i, end_i)

    key = (tuple(int(t) for t in tiles),)
    if key not in _CACHE:
        _CACHE.clear()
        _CACHE[key] = _build_bass(tiles, plan, t_total, len(plan))
    nc = _CACHE[key]

    in_maps = []
    for c in range(NCORES):
        colw, idx, wts = _host_prep(
            begin_i[asm[c]], end_i[asm[c]], tiles, plan, t_total
        )
        in_maps.append(
            {"seq": seq[asm[c].reshape(-1)], "colw": colw, "gidx": idx,
             "wts": wts}
        )

    LAST_SPMD = (nc, in_maps)
    last_exc = None
    out = np.empty((B, D), dtype=np.float32)
    for attempt in range(3):
        try:
            LAST_RESULTS = run_bass_kernel_spmd(
                nc, in_maps, core_ids=list(range(NCORES))
            )
            # materialize inside the retry: transient axon exec errors can
            # surface only at device->host fetch time
            for c in range(NCORES):
                out[asm[c].reshape(-1)] = np.asarray(
                    LAST_RESULTS.results[c]["outn"]
                )
            break
        except Exception as e:  # noqa: BLE001
            last_exc = e
            time.sleep(10.0)
    else:
        raise last_exc
    return out
